# revision 84
# baseline (speedup 1.0000x reference)
"""AttnBlock (GroupNorm + single-head self-attention + residual) on 8 trn2 cores.

Problem: x[8, 512, 2048]; per batch element:
    h = GroupNorm32(x) * gn_scale + gn_bias
    q/k/v = w{q,k,v} @ h + b  (1x1 conv == channel matmul), layout [L, C]
    w = softmax(q k^T / sqrt(C)); a = w v
    out = x + (wo @ a^T + bo)

Sharding: pure data-parallel over batch (B=8 == 8 cores), one batch element
per NeuronCore; weights replicated. No collectives.

Per-core layout strategy (C=512 -> 4 partition tiles, L=2048):
  - h, Q^T, K^T kept as [C-part, L-free]; Vl computed as [L-part, C-free]
  - S^T tile [k-part, q-free] = (K^T chunk).T @ Q^T  -> softmax denominator is
    a column sum == ones-matmul; no P transposes anywhere
  - A^T [C-part, q-free] = Vl.T @ P^T feeds the out-projection directly
  - softmax without max-subtraction (logits are ~N(0,1); |logit| < ~8)

All matmul operands are FP8 e4m3 run in DoubleRow perf mode: operand tiles
hold two 128-deep contraction subtiles side by side in the free dim (viewed
[128, 2, free]) and one instruction contracts all 256 rows at 2 rows/cycle,
halving PE instruction time vs fp32r.  PSUM accumulation stays fp32.  The
softmax P tile is scaled by 1/32 (folded into the exp bias) so the
unnormalized A = P V fits e4m3's finite range; the denominator is summed
from the same quantized P tile so the normalization A/d cancels the scale.
The residual path stays fp32 (x resident in SBUF), so fp8 only perturbs the
small attention delta (~4% of output norm); end-to-end rel err ~5e-3.

Engine budget notes: the Pool engine cannot touch PSUM on real HW (BIR
verifier enforces this), so every PSUM evacuation runs on ACT or DVE;
Pool gets the SBUF-only work (GN apply, final residual add).  ACT's exp
chain is the attention-phase bottleneck, so evacuations there lean DVE.
"""

import sys
import numpy as np

if "/opt/trn_rl_repo" not in sys.path:
    sys.path.insert(0, "/opt/trn_rl_repo")

import concourse.bass as bass
import concourse.bacc as bacc
import concourse.mybir as mybir
from concourse import tile

FP32 = mybir.dt.float32
FP8 = mybir.dt.float8e4

C = 512
L = 2048
G = 32
CPG = C // G  # 16 channels per group
EPS = 1e-5
NCT = C // 128  # 4 channel tiles
NPR = NCT // 2  # 2 channel-tile pairs (DoubleRow contraction units)
NLT = L // 128  # 16 L tiles
NQC = L // 512  # 4 q chunks
SCALE = float(np.float32(1.0) / np.sqrt(np.float32(C)))
# P = exp(s/sqrt(C) + LOG_PSCL) = exp(s/sqrt(C)) / 32: keeps the unnormalized
# A = P V inside fp8 e4m3 range; cancels in A/d.  NOTE mybir float8e4 is the
# IEEE e4m3 variant: max FINITE value 240, then +-inf (not the fn variant's
# 448-saturating behavior).  |A| reaches ~540 at 1/8 scaling on real inputs
# (-> inf -> NaN); at 1/32 it peaks ~135 with p <= ~29.
LOG_PSCL = float(np.log(1.0 / 32.0))

DR = mybir.MatmulPerfMode.DoubleRow
# dummy fp32 matmuls (1024 PE cycles each) burned at the start of each rep
# to hold the PE at full clock through the DMA/GN-stats idle window
NWARM = 12


def _pair(t):
    """View a [128, 2*F] tile as [128, 2, F] for DoubleRow operand slicing."""
    return t.rearrange("p (two f) -> p two f", two=2)


def build_kernel(nc, reps=1):
    x_d = nc.declare_dram_parameter("x", [C, L], FP32, isOutput=False)
    gns_d = nc.declare_dram_parameter("gn_scale", [C], FP32, isOutput=False)
    gnb_d = nc.declare_dram_parameter("gn_bias", [C], FP32, isOutput=False)
    wq_d = nc.declare_dram_parameter("wq", [C, C], FP32, isOutput=False)
    bq_d = nc.declare_dram_parameter("bq", [C], FP32, isOutput=False)
    wk_d = nc.declare_dram_parameter("wk", [C, C], FP32, isOutput=False)
    bk_d = nc.declare_dram_parameter("bk", [C], FP32, isOutput=False)
    wv_d = nc.declare_dram_parameter("wv", [C, C], FP32, isOutput=False)
    bv_d = nc.declare_dram_parameter("bv", [C], FP32, isOutput=False)
    wo_d = nc.declare_dram_parameter("wo", [C, C], FP32, isOutput=False)
    bo_d = nc.declare_dram_parameter("bo", [C], FP32, isOutput=False)
    out_d = nc.declare_dram_parameter("out", [C, L], FP32, isOutput=True)

    with tile.TileContext(nc) as tc:
        for _ in range(reps):
            _body(nc, tc, x_d, gns_d, gnb_d, wq_d, bq_d, wk_d, bk_d,
                  wv_d, bv_d, wo_d, bo_d, out_d)
    return nc


def _body(nc, tc, x_d, gns_d, gnb_d, wq_d, bq_d, wk_d, bk_d,
          wv_d, bv_d, wo_d, bo_d, out_d):
    from contextlib import ExitStack

    Id = mybir.ActivationFunctionType.Identity
    Exp = mybir.ActivationFunctionType.Exp
    Sqrt = mybir.ActivationFunctionType.Sqrt
    add = mybir.AluOpType.add
    mult = mybir.AluOpType.mult
    sub = mybir.AluOpType.subtract

    with ExitStack() as ctx:
        consts = ctx.enter_context(tc.tile_pool(name="consts", bufs=1))
        vecs = ctx.enter_context(tc.tile_pool(name="vecs", bufs=1))
        qt_pool = ctx.enter_context(tc.tile_pool(name="qt", bufs=NPR))
        kt_pool = ctx.enter_context(tc.tile_pool(name="kt", bufs=NPR))
        vl_pool = ctx.enter_context(tc.tile_pool(name="vl", bufs=NLT // 2))
        wot_pool = ctx.enter_context(tc.tile_pool(name="wot", bufs=NPR))
        # x stays resident in SBUF for the whole body: the GN stats/apply
        # read it and the final residual add reuses slices of it, so the
        # attention loop never re-reads x from HBM.
        xp = ctx.enter_context(tc.tile_pool(name="xp", bufs=NCT))

        # ---- constants ----
        onesf = consts.tile([128, 256], FP32, tag="onesf")
        nc.vector.memset(onesf[:], 1.0)
        ones8 = consts.tile([128, 256], FP8, tag="ones8")
        nc.vector.tensor_copy(ones8[:], onesf[:])
        logp_t = consts.tile([128, 1], FP32, tag="logp")
        nc.vector.memset(logp_t[:], LOG_PSCL)
        ident = consts.tile([128, 128], FP32, tag="ident")
        nc.vector.memset(ident[:], 1.0)
        # keep where (i - p) == 0
        nc.gpsimd.affine_select(ident[:], ident[:], [[1, 128]],
                                mybir.AluOpType.is_equal, 0.0,
                                base=0, channel_multiplier=-1)
        # group indicator Ind[p, g] = 1 iff p//16 == g   (iota = p - 16 g)
        ind = consts.tile([128, G // 4], FP32, tag="ind")  # [128, 8]
        nc.vector.memset(ind[:], 1.0)
        nc.gpsimd.affine_select(ind[:], ind[:], [[-CPG, G // 4]],
                                mybir.AluOpType.is_ge, 0.0,
                                base=0, channel_multiplier=1)
        nc.gpsimd.affine_select(ind[:], ind[:], [[CPG, G // 4]],
                                mybir.AluOpType.is_ge, 0.0,
                                base=CPG - 1, channel_multiplier=-1)
        # J[g, p] = 1 iff p//16 == g  (iota = p - 16 g)
        jmat = consts.tile([G // 4, 128], FP32, tag="jmat")  # [8, 128]
        nc.vector.memset(jmat[:], 1.0)
        nc.gpsimd.affine_select(jmat[:], jmat[:], [[1, 128]],
                                mybir.AluOpType.is_ge, 0.0,
                                base=0, channel_multiplier=-CPG)
        nc.gpsimd.affine_select(jmat[:], jmat[:], [[-1, 128]],
                                mybir.AluOpType.is_ge, 0.0,
                                base=CPG - 1, channel_multiplier=CPG)

        # ---- per-partition vectors: [512] -> [128, 4] in ONE DMA (the DMA
        # queue head costs ~1.6us per instruction, so batch aggressively);
        # t[p, i] = vec[128 i + p].  Issued on the ACT-side HWDGE queue.
        def load_vec(dram, name):
            t = vecs.tile([128, NCT], FP32, tag=name, name=name + "_sb")
            nc.scalar.dma_start(out=t[:],
                                in_=dram[:].rearrange("(f p) -> p f", p=128))
            return t

        # fp8 DoubleRow pair tiles, all [128, 2*F] with the two 128-deep
        # contraction subtiles in the free-dim halves:
        #   qt2/kt2[j]: Q^T/K^T c-tiles (2j, 2j+1), F = L
        #   vl2[m]:     V l-tiles (2m, 2m+1), F = C
        #   wo2T[j]:    wo^T c-in tiles (2j, 2j+1), F = C
        qt2, kt2, vl2, wo2T = [], [], [], []

        with ExitStack() as setup_ctx:
            wsb = setup_ctx.enter_context(tc.tile_pool(name="wsb", bufs=2))
            hp = setup_ctx.enter_context(tc.tile_pool(name="hp", bufs=NPR))
            gn_sb = setup_ctx.enter_context(tc.tile_pool(name="gnsb", bufs=1))
            wqT_pool = setup_ctx.enter_context(tc.tile_pool(name="wqT", bufs=NPR))
            wkT_pool = setup_ctx.enter_context(tc.tile_pool(name="wkT", bufs=NPR))
            wvT_pool = setup_ctx.enter_context(tc.tile_pool(name="wvT", bufs=NPR))
            ps_t = setup_ctx.enter_context(
                tc.tile_pool(name="ps_t", bufs=2, space="PSUM"))
            ps_gn = setup_ctx.enter_context(
                tc.tile_pool(name="ps_gn", bufs=1, space="PSUM"))
            ps_p = setup_ctx.enter_context(
                tc.tile_pool(name="ps_p", bufs=2, space="PSUM"))

            # ---- PE clock warm-up: the PE idles ~10us at the start of each
            # rep (x DMA + GN stats) and drops to a low p-state, so the
            # first ~3us of real matmuls would run at half clock.  Burn the
            # idle window with discarded dummy matmuls (no data deps) so the
            # transposes/projections start at full speed. ----
            for wi in range(NWARM):
                wps = ps_t.tile([128, 256], FP32, tag="tp", name="warm")
                nc.tensor.matmul(wps[:], ident[:], onesf[:],
                                 start=True, stop=True)

            # ---- load x first (it gates GN -> h -> everything); stats as
            # each tile lands: sums on DVE/Pool, sum-of-squares on ACT ----
            h2 = [hp.tile([128, 2 * L], FP8, tag="hp", name=f"h{j}")
                  for j in range(NPR)]
            sqscr = gn_sb.tile([128, L], FP32, tag="sqscr")
            stats = gn_sb.tile([128, 2 * NCT], FP32, tag="stats")
            xtiles = {}
            Square = mybir.ActivationFunctionType.Square
            for t in range(NCT):
                x_t = xp.tile([128, L], FP32, tag="xp", name=f"x{t}")
                xtiles[t] = x_t
                # alternate the two HWDGE queues (SP / ACT-issued) so the
                # four x tiles stream in parallel pairs
                dma_eng = nc.sync if t % 2 == 0 else nc.scalar
                dma_eng.dma_start(out=x_t[:], in_=x_d[128 * t:128 * (t + 1), :])
                nc.vector.tensor_reduce(stats[:, 2 * t:2 * t + 1], x_t[:],
                                        mybir.AxisListType.X, add)
                # Square output is dumped to fp32 scratch; only the fp32
                # accumulator (sum of squares) matters
                nc.scalar.activation(sqscr[:], x_t[:], Square,
                                     accum_out=stats[:, 2 * t + 1:2 * t + 2])

            # small per-partition vectors on the ACT-side queue behind x1/x3
            gns_t = load_vec(gns_d, "gns")
            gnb_t = load_vec(gnb_d, "gnb")
            bq_t = load_vec(bq_d, "bq")
            bk_t = load_vec(bk_d, "bk")
            bv_t = load_vec(bv_d, "bv")
            bo_t = load_vec(bo_d, "bo")

            # ---- weight transposes (PE), one DMA per weight ([512,512]
            # loaded as [128, 4*512]: w[u*128+p, c] -> t[p, u*512+c]).
            # The four [128,128] transposes of one output row-block
            # accumulate into disjoint slices of one [128,512] PSUM bank
            # (start only on the first: a start flag zeroes the whole 2KB
            # zero-region), then ONE merged evacuation copy moves it to the
            # fp8 pair tile. ----
            wq2T, wk2T, wv2T = [], [], []

            def transpose_weight(w_d, nm, pool, dst, dma_eng):
                for j in range(NPR):
                    dst.append(pool.tile([128, 2 * C], FP8,
                                         tag=pool.name, name=f"{nm}T{j}"))
                w_t = wsb.tile([128, NCT * C], FP32, tag="wsb", name=f"w{nm}")
                dma_eng.dma_start(
                    out=w_t[:].rearrange("p (u c) -> p u c", u=NCT),
                    in_=w_d[:].rearrange("(u p) c -> p u c", p=128))
                for t in range(NCT):
                    tp = ps_t.tile([128, 512], FP32, tag="tp", name="tp")
                    for u in range(NCT):
                        nc.tensor.matmul(
                            tp[:, 128 * u:128 * (u + 1)],
                            w_t[:, u * C + 128 * t:u * C + 128 * (t + 1)],
                            ident[:], is_transpose=True,
                            start=(u == 0), stop=(u == NCT - 1),
                            skip_group_check=True)
                    half = (t % 2) * C
                    dst_ap = dst[t // 2][:, half:half + C]
                    if t % 2 == 0:
                        nc.vector.tensor_copy(dst_ap, tp[:])
                    else:
                        nc.scalar.copy(dst_ap, tp[:])

            # ---- finish GN stats: group reduce + broadcast ----
            inv_n = float(1.0 / (CPG * L))
            gsum_ps = ps_gn.tile([G // 4, 2 * NCT], FP32, tag="gsum")
            nc.tensor.matmul(gsum_ps[:], ind[:], stats[:])
            # mr[:, 0:4] = mean, mr[:, 4:8] = rstd   per c-tile column
            mr = gn_sb.tile([G // 4, 2 * NCT], FP32, tag="mr")
            tmp8 = gn_sb.tile([G // 4, NCT], FP32, tag="tmp8")
            gview = gsum_ps.rearrange("p (c two) -> p c two", two=2)
            nc.vector.tensor_scalar_mul(mr[:, 0:NCT], gview[:, :, 0], inv_n)
            nc.vector.tensor_scalar_mul(tmp8[:], gview[:, :, 1], inv_n)
            # var = E[x^2] - mean^2 ; rstd = 1/sqrt(var + eps)
            var8 = gn_sb.tile([G // 4, NCT], FP32, tag="var8")
            nc.vector.tensor_tensor(var8[:], mr[:, 0:NCT], mr[:, 0:NCT], mult)
            nc.vector.tensor_tensor(var8[:], tmp8[:], var8[:], sub)
            std8 = gn_sb.tile([G // 4, NCT], FP32, tag="std8")
            eps8 = gn_sb.tile([G // 4, 1], FP32, tag="eps8")
            nc.vector.memset(eps8[:], EPS)
            nc.scalar.activation(std8[:], var8[:], Sqrt, bias=eps8[:])
            nc.vector.reciprocal(mr[:, NCT:2 * NCT], std8[:])

            # broadcast mean/rstd to per-partition, apply GN affine on the
            # Pool engine (SBUF-only op: h = x * a + b via two scalars)
            mr_v = mr.rearrange("p (h f) -> p h f", h=2)
            for t in range(NCT):
                bc = ps_t.tile([128, 512], FP32, tag="tp", name=f"bc{t}")
                nc.tensor.matmul(bc[:, 0:2], jmat[:], mr_v[:, :, t])
                a_t = gn_sb.tile([128, 1], FP32, tag=f"a{t}", name=f"a{t}")
                b_t = gn_sb.tile([128, 1], FP32, tag=f"b{t}", name=f"b{t}")
                nc.vector.tensor_tensor(a_t[:], bc[:, 1:2], gns_t[:, t:t + 1],
                                        mult)
                # b = gn_bias - mean * a
                nc.vector.tensor_tensor(b_t[:], bc[:, 0:1], a_t[:], mult)
                nc.vector.tensor_tensor(b_t[:], gnb_t[:, t:t + 1], b_t[:], sub)
                # split the applies across DVE/ACT so h2 is ready in ~2
                # apply-times (Pool cannot run AP-scalar ops)
                h2_ap = h2[t // 2][:, (t % 2) * L:(t % 2 + 1) * L]
                if t % 2 == 0:
                    nc.vector.tensor_scalar(h2_ap, xtiles[t][:],
                                            a_t[:], b_t[:], mult, add)
                else:
                    nc.scalar.activation(h2_ap, xtiles[t][:], Id,
                                         bias=b_t[:], scale=a_t[:])

            transpose_weight(wq_d, "wq", wqT_pool, wq2T, nc.sync)
            transpose_weight(wk_d, "wk", wkT_pool, wk2T, nc.sync)
            transpose_weight(wv_d, "wv", wvT_pool, wv2T, nc.scalar)

            # ---- projections (all DoubleRow fp8: contract c-pairs).
            # Two adjacent 512-wide outputs accumulate into the two banks of
            # one [128,1024] PSUM tile; a single instruction evacuates both,
            # alternating DVE/ACT. ----
            for dst_list, w2T, bvec, pool, nmo in (
                    (qt2, wq2T, bq_t, qt_pool, "q"),
                    (kt2, wk2T, bk_t, kt_pool, "k")):
                for j in range(NPR):
                    dst_list.append(pool.tile([128, 2 * L], FP8, tag=pool.name,
                                              name=f"{nmo}T{j}"))
                for t in range(NCT):
                    half = (t % 2) * L
                    for lc2 in range(NQC // 2):
                        pp = ps_p.tile([128, 1024], FP32, tag="pp", name="pp")
                        for sub_i in range(2):
                            lc = 2 * lc2 + sub_i
                            for j in range(NPR):
                                nc.tensor.matmul(
                                    pp[:, 512 * sub_i:512 * (sub_i + 1)],
                                    _pair(w2T[j])[:, 0:2, 128 * t:128 * (t + 1)],
                                    _pair(h2[j])[:, 0:2,
                                                 512 * lc:512 * (lc + 1)],
                                    start=(j == 0), stop=(j == NPR - 1),
                                    perf_mode=DR, skip_group_check=True)
                        dst_ap = dst_list[t // 2][:, half + 1024 * lc2:
                                                  half + 1024 * (lc2 + 1)]
                        if (t * 2 + lc2) % 2 == 0:
                            nc.scalar.activation(dst_ap, pp[:], Id,
                                                 bias=bvec[:, t:t + 1])
                        else:
                            nc.vector.tensor_scalar(dst_ap, pp[:],
                                                    bvec[:, t:t + 1], None, add)

            # V: bv is NOT added here — a = (P(V + 1 bv^T))/d = PV/d + bv,
            # so bv folds into the out-projection bias: bo' = bo + wo @ bv
            # (computed below with two tiny DoubleRow matmuls per c-tile).
            for m in range(NLT // 2):
                vl2.append(vl_pool.tile([128, 2 * C], FP8, tag="vl",
                                        name=f"vl{m}"))
            for lt2 in range(NLT // 2):
                pp = ps_p.tile([128, 1024], FP32, tag="pp", name="pp")
                for sub_i in range(2):
                    lt = 2 * lt2 + sub_i
                    for j in range(NPR):
                        nc.tensor.matmul(
                            pp[:, 512 * sub_i:512 * (sub_i + 1)],
                            _pair(h2[j])[:, 0:2, 128 * lt:128 * (lt + 1)],
                            _pair(wv2T[j])[:, 0:2, :],
                            start=(j == 0), stop=(j == NPR - 1),
                            perf_mode=DR, skip_group_check=True)
                if lt2 % 2 == 0:
                    nc.scalar.copy(vl2[lt2][:], pp[:])
                else:
                    nc.vector.tensor_copy(vl2[lt2][:], pp[:])

            # wo^T is not needed until the first out-projection (~end of the
            # first attention q-chunk) — transpose it last
            transpose_weight(wo_d, "wo", wot_pool, wo2T, nc.scalar)

            # bo' = bo + wo @ bv  (8 tiny DoubleRow matmuls into one bank)
            bv8 = gn_sb.tile([128, NCT], FP8, tag="bv8")
            nc.vector.tensor_copy(bv8[:], bv_t[:])
            wob_ps = ps_gn.tile([128, NCT], FP32, tag="wob")
            for ot in range(NCT):
                for j in range(NPR):
                    bv_pair = bv8[:, 2 * j:2 * j + 2].rearrange(
                        "p (two f) -> p two f", two=2)
                    nc.tensor.matmul(
                        wob_ps[:, ot:ot + 1],
                        _pair(wo2T[j])[:, 0:2, 128 * ot:128 * (ot + 1)],
                        bv_pair[:, 0:2, :],
                        start=(ot == 0 and j == 0),
                        stop=(ot == NCT - 1 and j == NPR - 1),
                        perf_mode=DR, skip_group_check=True)
            bo2_t = vecs.tile([128, NCT], FP32, tag="bo2")
            nc.vector.tensor_tensor(bo2_t[:], wob_ps[:], bo_t[:], add)

        # ---- attention ----
        with ExitStack() as att_ctx:
            pt_pool = att_ctx.enter_context(tc.tile_pool(name="pt", bufs=6))
            dinv_pool = att_ctx.enter_context(tc.tile_pool(name="dinv", bufs=2))
            asb_pool = att_ctx.enter_context(tc.tile_pool(name="asb", bufs=4))
            osb_pool = att_ctx.enter_context(tc.tile_pool(name="osb", bufs=4))
            # 8 PSUM banks: 3 x S (two-deep prefetch) + 2x2 x A + 1 x d.
            # The out-projection o_ps tiles borrow freed A banks (same pool)
            # after the A evacuation each q-chunk.
            ps_s = att_ctx.enter_context(
                tc.tile_pool(name="ps_s", bufs=3, space="PSUM"))
            ps_a = att_ctx.enter_context(
                tc.tile_pool(name="ps_a", bufs=2, space="PSUM"))
            ps_d = att_ctx.enter_context(
                tc.tile_pool(name="ps_d", bufs=1, space="PSUM"))

            def s_block(qc_i, kt_i):
                s_tile = ps_s.tile([128, 512], FP32, tag="s",
                                   name=f"s{qc_i}_{kt_i}")
                for j in range(NPR):
                    nc.tensor.matmul(
                        s_tile[:],
                        _pair(kt2[j])[:, 0:2, 128 * kt_i:128 * (kt_i + 1)],
                        _pair(qt2[j])[:, 0:2, 512 * qc_i:512 * qc_i + 512],
                        start=(j == 0), stop=(j == NPR - 1), perf_mode=DR)
                return s_tile

            from collections import deque
            s_ahead = deque()
            s_ahead.append(s_block(0, 0))
            s_ahead.append(s_block(0, 1))
            for qc in range(NQC):
                q0 = 512 * qc
                # a_ps[j] holds c-tiles (2j, 2j+1) in its two banks
                a_ps = [ps_a.tile([128, 1024], FP32, tag="a", name=f"a_ps{j}")
                        for j in range(NPR)]
                d_ps = ps_d.tile([128, 512], FP32, tag="d", name="d_ps")
                p_cur = None
                for kt_i in range(NLT):
                    if kt_i % 2 == 0:
                        p_cur = pt_pool.tile([128, 1024], FP8, tag="pt",
                                             name="p_t")
                    s_cur = s_ahead.popleft()
                    nc.scalar.activation(
                        p_cur[:, (kt_i % 2) * 512:(kt_i % 2 + 1) * 512],
                        s_cur[:], Exp, scale=SCALE, bias=logp_t[:])
                    # keep two S blocks in flight ahead of the exp so ACT
                    # never waits on the PE
                    nxt = kt_i + 2
                    if nxt < NLT:
                        s_ahead.append(s_block(qc, nxt))
                    elif qc + 1 < NQC:
                        s_ahead.append(s_block(qc + 1, nxt - NLT))
                    if kt_i % 2 == 0:
                        continue
                    pair = kt_i // 2
                    first = pair == 0
                    last = pair == NLT // 2 - 1
                    pv = _pair(p_cur)
                    for cc in range(NCT):
                        nc.tensor.matmul(
                            a_ps[cc // 2][:, (cc % 2) * 512:(cc % 2 + 1) * 512],
                            _pair(vl2[pair])[:, 0:2, 128 * cc:128 * (cc + 1)],
                            pv[:, 0:2, :],
                            start=first, stop=last, skip_group_check=True,
                            perf_mode=DR)
                    # softmax denominator: ones-matmul accumulation (the
                    # result lands broadcast across all 128 partitions)
                    nc.tensor.matmul(d_ps[:], _pair(ones8)[:, 0:2, :],
                                     pv[:, 0:2, :],
                                     start=first, stop=last,
                                     skip_group_check=True, perf_mode=DR)

                # Evacuate UNNORMALIZED A (frees the accumulation banks
                # without waiting for the reciprocal); the softmax division
                # commutes with the out-projection (per-column scaling), so
                # it is applied at the final evacuation instead.  Both
                # merged copies on DVE — ACT stays dedicated to the exps.
                a2 = []
                for j in range(NPR):
                    t = asb_pool.tile([128, 1024], FP8, tag="asb",
                                      name=f"asb{j}")
                    # ACT takes one (it idles at the q-chunk boundary)
                    if j == 0:
                        nc.scalar.copy(t[:], a_ps[j][:])
                    else:
                        nc.vector.tensor_copy(t[:], a_ps[j][:])
                    a2.append(t)
                dinv = dinv_pool.tile([128, 512], FP32, tag="dinv", name="dinv")
                dscr = dinv_pool.tile([128, 512], FP32, tag="dscr", name="dscr")
                nc.vector.reciprocal_approx_accurate(out=dinv[:], in_=d_ps[:],
                                                     scratch=dscr[:])

                for op in range(NPR):  # two [128,1024] out tiles: ot pairs
                    o_ps = ps_a.tile([128, 1024], FP32, tag="a", name="o_ps")
                    for sub_i in range(2):
                        ot = 2 * op + sub_i
                        for j in range(NPR):
                            nc.tensor.matmul(
                                o_ps[:, 512 * sub_i:512 * (sub_i + 1)],
                                _pair(wo2T[j])[:, 0:2, 128 * ot:128 * (ot + 1)],
                                _pair(a2[j])[:, 0:2, :],
                                start=(j == 0), stop=(j == NPR - 1),
                                perf_mode=DR, skip_group_check=True)
                    osb = osb_pool.tile([128, 1024], FP32, tag="osb",
                                        name="osb")
                    for sub_i in range(2):
                        ot = 2 * op + sub_i
                        sl = slice(512 * sub_i, 512 * (sub_i + 1))
                        tmp = osb_pool.tile([128, 512], FP32, tag="otmp",
                                            name="otmp")
                        nc.vector.tensor_tensor(tmp[:], o_ps[:, sl],
                                                dinv[:], mult)
                        nc.vector.scalar_tensor_tensor(
                            osb[:, sl], tmp[:], bo2_t[:, ot:ot + 1],
                            xtiles[ot][:, q0:q0 + 512],
                            op0=add, op1=add)
                    # one DMA for both ot row-blocks (256 DRAM rows);
                    # alternate the two queues
                    dma_eng = nc.scalar if op % 2 == 0 else nc.sync
                    dma_eng.dma_start(
                        out=out_d[256 * op:256 * (op + 1), q0:q0 + 512]
                        .rearrange("(two p) c -> p two c", p=128),
                        in_=osb[:].rearrange("p (two c) -> p two c", two=2))


def make_nc():
    return bacc.Bacc("TRN2", target_bir_lowering=False, debug=False)


_NC_CACHE = []


def kernel(**inputs):
    from concourse.bass_utils import run_bass_kernel_spmd

    x = np.ascontiguousarray(inputs["x"], dtype=np.float32)
    B = x.shape[0]
    assert B == 8, f"kernel is built for B=8 (one batch element per core), got {B}"
    shared = {}
    for name in ("gn_scale", "gn_bias", "wq", "bq", "wk", "bk",
                 "wv", "bv", "wo", "bo"):
        shared[name] = np.ascontiguousarray(inputs[name], dtype=np.float32)

    if not _NC_CACHE:
        nc = make_nc()
        build_kernel(nc)
        nc.compile()
        _NC_CACHE.append(nc)
    nc = _NC_CACHE[0]

    core_ids = list(range(B))
    in_maps = [dict(shared, x=x[i]) for i in range(B)]
    res = run_bass_kernel_spmd(nc, in_maps, core_ids)
    out = np.stack([res.results[i]["out"] for i in range(B)], axis=0)
    return out.astype(np.float32)


if __name__ == "__main__":
    rng = np.random.default_rng(0)
    demo = {
        "x": rng.standard_normal((8, C, L), dtype=np.float32),
        "gn_scale": np.ones(C, np.float32),
        "gn_bias": np.zeros(C, np.float32),
    }
    for w, b in (("wq", "bq"), ("wk", "bk"), ("wv", "bv"), ("wo", "bo")):
        demo[w] = rng.standard_normal((C, C), dtype=np.float32) / np.sqrt(C)
        demo[b] = np.zeros(C, np.float32)
    out = kernel(**demo)
    print(out.shape, out.dtype)


# revision 85
# speedup vs baseline: 1.3378x; 1.3378x over previous
"""AttnBlock (GroupNorm + single-head self-attention + residual) on 8 trn2 cores.

Problem: x[8, 512, 2048]; per batch element:
    h = GroupNorm32(x) * gn_scale + gn_bias
    q/k/v = w{q,k,v} @ h + b  (1x1 conv == channel matmul), layout [L, C]
    w = softmax(q k^T / sqrt(C)); a = w v
    out = x + (wo @ a^T + bo)

Sharding: pure data-parallel over batch (B=8 == 8 cores), one batch element
per NeuronCore; weights replicated. No collectives.

Per-core layout strategy (C=512 -> 4 partition tiles, L=2048):
  - h, Q^T, K^T kept as [C-part, L-free]; Vl computed as [L-part, C-free]
  - S^T tile [k-part, q-free] = (K^T chunk).T @ Q^T  -> softmax denominator is
    a column sum == ones-matmul; no P transposes anywhere
  - A^T [C-part, q-free] = Vl.T @ P^T feeds the out-projection directly
  - softmax without max-subtraction (logits are ~N(0,1); |logit| < ~8)

All matmul operands are FP8 e4m3 run in DoubleRow perf mode: operand tiles
hold two 128-deep contraction subtiles side by side in the free dim (viewed
[128, 2, free]) and one instruction contracts all 256 rows at 2 rows/cycle,
halving PE instruction time vs fp32r.  PSUM accumulation stays fp32.  The
softmax P tile is scaled by 1/32 (folded into the exp bias) so the
unnormalized A = P V fits e4m3's finite range; the denominator is summed
from the same quantized P tile so the normalization A/d cancels the scale.
The residual path stays fp32 (x resident in SBUF), so fp8 only perturbs the
small attention delta (~4% of output norm); end-to-end rel err ~5e-3.

Engine budget notes: the Pool engine cannot touch PSUM on real HW (BIR
verifier enforces this), so every PSUM evacuation runs on ACT or DVE;
Pool gets the SBUF-only work (GN apply, final residual add).  ACT's exp
chain is the attention-phase bottleneck, so evacuations there lean DVE.
"""

import sys
import numpy as np

if "/opt/trn_rl_repo" not in sys.path:
    sys.path.insert(0, "/opt/trn_rl_repo")

import concourse.bass as bass
import concourse.bacc as bacc
import concourse.mybir as mybir
from concourse import tile

FP32 = mybir.dt.float32
FP8 = mybir.dt.float8e4

C = 512
L = 2048
G = 32
CPG = C // G  # 16 channels per group
EPS = 1e-5
NCT = C // 128  # 4 channel tiles
NPR = NCT // 2  # 2 channel-tile pairs (DoubleRow contraction units)
NLT = L // 128  # 16 L tiles
NQC = L // 512  # 4 q chunks
SCALE = float(np.float32(1.0) / np.sqrt(np.float32(C)))
# P = exp(s/sqrt(C) + LOG_PSCL) = exp(s/sqrt(C)) / 32: keeps the unnormalized
# A = P V inside fp8 e4m3 range; cancels in A/d.  NOTE mybir float8e4 is the
# IEEE e4m3 variant: max FINITE value 240, then +-inf (not the fn variant's
# 448-saturating behavior).  |A| reaches ~540 at 1/8 scaling on real inputs
# (-> inf -> NaN); at 1/32 it peaks ~135 with p <= ~29.
LOG_PSCL = float(np.log(1.0 / 32.0))

DR = mybir.MatmulPerfMode.DoubleRow


def _pair(t):
    """View a [128, 2*F] tile as [128, 2, F] for DoubleRow operand slicing."""
    return t.rearrange("p (two f) -> p two f", two=2)


def build_kernel(nc, reps=1):
    x_d = nc.declare_dram_parameter("x", [C, L], FP32, isOutput=False)
    gns_d = nc.declare_dram_parameter("gn_scale", [C], FP32, isOutput=False)
    gnb_d = nc.declare_dram_parameter("gn_bias", [C], FP32, isOutput=False)
    wq_d = nc.declare_dram_parameter("wq", [C, C], FP32, isOutput=False)
    bq_d = nc.declare_dram_parameter("bq", [C], FP32, isOutput=False)
    wk_d = nc.declare_dram_parameter("wk", [C, C], FP32, isOutput=False)
    bk_d = nc.declare_dram_parameter("bk", [C], FP32, isOutput=False)
    wv_d = nc.declare_dram_parameter("wv", [C, C], FP32, isOutput=False)
    bv_d = nc.declare_dram_parameter("bv", [C], FP32, isOutput=False)
    wo_d = nc.declare_dram_parameter("wo", [C, C], FP32, isOutput=False)
    bo_d = nc.declare_dram_parameter("bo", [C], FP32, isOutput=False)
    out_d = nc.declare_dram_parameter("out", [C, L], FP32, isOutput=True)

    with tile.TileContext(nc) as tc:
        for _ in range(reps):
            _body(nc, tc, x_d, gns_d, gnb_d, wq_d, bq_d, wk_d, bk_d,
                  wv_d, bv_d, wo_d, bo_d, out_d)
    return nc


def _body(nc, tc, x_d, gns_d, gnb_d, wq_d, bq_d, wk_d, bk_d,
          wv_d, bv_d, wo_d, bo_d, out_d):
    from contextlib import ExitStack

    Id = mybir.ActivationFunctionType.Identity
    Exp = mybir.ActivationFunctionType.Exp
    Sqrt = mybir.ActivationFunctionType.Sqrt
    add = mybir.AluOpType.add
    mult = mybir.AluOpType.mult
    sub = mybir.AluOpType.subtract

    with ExitStack() as ctx:
        consts = ctx.enter_context(tc.tile_pool(name="consts", bufs=1))
        vecs = ctx.enter_context(tc.tile_pool(name="vecs", bufs=1))
        qt_pool = ctx.enter_context(tc.tile_pool(name="qt", bufs=NPR))
        kt_pool = ctx.enter_context(tc.tile_pool(name="kt", bufs=NPR))
        vl_pool = ctx.enter_context(tc.tile_pool(name="vl", bufs=NLT // 2))
        wot_pool = ctx.enter_context(tc.tile_pool(name="wot", bufs=NPR))
        # x stays resident in SBUF for the whole body: the GN stats/apply
        # read it and the final residual add reuses slices of it, so the
        # attention loop never re-reads x from HBM.
        xp = ctx.enter_context(tc.tile_pool(name="xp", bufs=NCT))

        # ---- constants ----
        onesf = consts.tile([128, 256], FP32, tag="onesf")
        nc.vector.memset(onesf[:], 1.0)
        ones8 = consts.tile([128, 256], FP8, tag="ones8")
        nc.vector.tensor_copy(ones8[:], onesf[:])
        logp_t = consts.tile([128, 1], FP32, tag="logp")
        nc.vector.memset(logp_t[:], LOG_PSCL)
        ident = consts.tile([128, 128], FP32, tag="ident")
        nc.vector.memset(ident[:], 1.0)
        # keep where (i - p) == 0
        nc.gpsimd.affine_select(ident[:], ident[:], [[1, 128]],
                                mybir.AluOpType.is_equal, 0.0,
                                base=0, channel_multiplier=-1)
        # group indicator Ind[p, g] = 1 iff p//16 == g   (iota = p - 16 g)
        ind = consts.tile([128, G // 4], FP32, tag="ind")  # [128, 8]
        nc.vector.memset(ind[:], 1.0)
        nc.gpsimd.affine_select(ind[:], ind[:], [[-CPG, G // 4]],
                                mybir.AluOpType.is_ge, 0.0,
                                base=0, channel_multiplier=1)
        nc.gpsimd.affine_select(ind[:], ind[:], [[CPG, G // 4]],
                                mybir.AluOpType.is_ge, 0.0,
                                base=CPG - 1, channel_multiplier=-1)
        # J[g, p] = 1 iff p//16 == g  (iota = p - 16 g)
        jmat = consts.tile([G // 4, 128], FP32, tag="jmat")  # [8, 128]
        nc.vector.memset(jmat[:], 1.0)
        nc.gpsimd.affine_select(jmat[:], jmat[:], [[1, 128]],
                                mybir.AluOpType.is_ge, 0.0,
                                base=0, channel_multiplier=-CPG)
        nc.gpsimd.affine_select(jmat[:], jmat[:], [[-1, 128]],
                                mybir.AluOpType.is_ge, 0.0,
                                base=CPG - 1, channel_multiplier=CPG)

        # ---- per-partition vectors: [512] -> [128, 4] in ONE DMA (the DMA
        # queue head costs ~1.6us per instruction, so batch aggressively);
        # t[p, i] = vec[128 i + p].  Issued on the ACT-side HWDGE queue.
        def load_vec(dram, name):
            t = vecs.tile([128, NCT], FP32, tag=name, name=name + "_sb")
            nc.scalar.dma_start(out=t[:],
                                in_=dram[:].rearrange("(f p) -> p f", p=128))
            return t

        # fp8 DoubleRow pair tiles, all [128, 2*F] with the two 128-deep
        # contraction subtiles in the free-dim halves:
        #   qt2/kt2[j]: Q^T/K^T c-tiles (2j, 2j+1), F = L
        #   vl2[m]:     V l-tiles (2m, 2m+1), F = C
        #   wo2T[j]:    wo^T c-in tiles (2j, 2j+1), F = C
        qt2, kt2, vl2, wo2T = [], [], [], []

        with ExitStack() as setup_ctx:
            wsb = setup_ctx.enter_context(tc.tile_pool(name="wsb", bufs=2))
            hp = setup_ctx.enter_context(tc.tile_pool(name="hp", bufs=NPR))
            gn_sb = setup_ctx.enter_context(tc.tile_pool(name="gnsb", bufs=1))
            wqT_pool = setup_ctx.enter_context(tc.tile_pool(name="wqT", bufs=NPR))
            wkT_pool = setup_ctx.enter_context(tc.tile_pool(name="wkT", bufs=NPR))
            wvT_pool = setup_ctx.enter_context(tc.tile_pool(name="wvT", bufs=NPR))
            ps_t = setup_ctx.enter_context(
                tc.tile_pool(name="ps_t", bufs=2, space="PSUM"))
            ps_gn = setup_ctx.enter_context(
                tc.tile_pool(name="ps_gn", bufs=1, space="PSUM"))
            ps_p = setup_ctx.enter_context(
                tc.tile_pool(name="ps_p", bufs=2, space="PSUM"))

            # ---- load x first (it gates GN -> h -> everything); stats as
            # each tile lands: sums on DVE/Pool, sum-of-squares on ACT ----
            h2 = [hp.tile([128, 2 * L], FP8, tag="hp", name=f"h{j}")
                  for j in range(NPR)]
            sqscr = gn_sb.tile([128, L], FP32, tag="sqscr")
            stats = gn_sb.tile([128, 2 * NCT], FP32, tag="stats")
            xtiles = {}
            Square = mybir.ActivationFunctionType.Square
            for t in range(NCT):
                x_t = xp.tile([128, L], FP32, tag="xp", name=f"x{t}")
                xtiles[t] = x_t
                # alternate the two HWDGE queues (SP / ACT-issued) so the
                # four x tiles stream in parallel pairs
                dma_eng = nc.sync if t % 2 == 0 else nc.scalar
                dma_eng.dma_start(out=x_t[:], in_=x_d[128 * t:128 * (t + 1), :])
                nc.vector.tensor_reduce(stats[:, 2 * t:2 * t + 1], x_t[:],
                                        mybir.AxisListType.X, add)
                # Square output is dumped to fp32 scratch; only the fp32
                # accumulator (sum of squares) matters
                nc.scalar.activation(sqscr[:], x_t[:], Square,
                                     accum_out=stats[:, 2 * t + 1:2 * t + 2])

            # small per-partition vectors on the ACT-side queue behind x1/x3
            gns_t = load_vec(gns_d, "gns")
            gnb_t = load_vec(gnb_d, "gnb")
            bq_t = load_vec(bq_d, "bq")
            bk_t = load_vec(bk_d, "bk")
            bv_t = load_vec(bv_d, "bv")
            bo_t = load_vec(bo_d, "bo")

            # ---- weight transposes (PE), one DMA per weight ([512,512]
            # loaded as [128, 4*512]: w[u*128+p, c] -> t[p, u*512+c]).
            # The four [128,128] transposes of one output row-block
            # accumulate into disjoint slices of one [128,512] PSUM bank
            # (start only on the first: a start flag zeroes the whole 2KB
            # zero-region), then ONE merged evacuation copy moves it to the
            # fp8 pair tile. ----
            wq2T, wk2T, wv2T = [], [], []

            def transpose_weight(w_d, nm, pool, dst, dma_eng):
                for j in range(NPR):
                    dst.append(pool.tile([128, 2 * C], FP8,
                                         tag=pool.name, name=f"{nm}T{j}"))
                w_t = wsb.tile([128, NCT * C], FP32, tag="wsb", name=f"w{nm}")
                dma_eng.dma_start(
                    out=w_t[:].rearrange("p (u c) -> p u c", u=NCT),
                    in_=w_d[:].rearrange("(u p) c -> p u c", p=128))
                for t in range(NCT):
                    tp = ps_t.tile([128, 512], FP32, tag="tp", name="tp")
                    for u in range(NCT):
                        nc.tensor.matmul(
                            tp[:, 128 * u:128 * (u + 1)],
                            w_t[:, u * C + 128 * t:u * C + 128 * (t + 1)],
                            ident[:], is_transpose=True,
                            start=(u == 0), stop=(u == NCT - 1),
                            skip_group_check=True)
                    half = (t % 2) * C
                    dst_ap = dst[t // 2][:, half:half + C]
                    if t % 2 == 0:
                        nc.vector.tensor_copy(dst_ap, tp[:])
                    else:
                        nc.scalar.copy(dst_ap, tp[:])

            # ---- finish GN stats: group reduce + broadcast ----
            inv_n = float(1.0 / (CPG * L))
            gsum_ps = ps_gn.tile([G // 4, 2 * NCT], FP32, tag="gsum")
            nc.tensor.matmul(gsum_ps[:], ind[:], stats[:])
            # mr[:, 0:4] = mean, mr[:, 4:8] = rstd   per c-tile column
            mr = gn_sb.tile([G // 4, 2 * NCT], FP32, tag="mr")
            tmp8 = gn_sb.tile([G // 4, NCT], FP32, tag="tmp8")
            gview = gsum_ps.rearrange("p (c two) -> p c two", two=2)
            nc.vector.tensor_scalar_mul(mr[:, 0:NCT], gview[:, :, 0], inv_n)
            nc.vector.tensor_scalar_mul(tmp8[:], gview[:, :, 1], inv_n)
            # var = E[x^2] - mean^2 ; rstd = 1/sqrt(var + eps)
            var8 = gn_sb.tile([G // 4, NCT], FP32, tag="var8")
            nc.vector.tensor_tensor(var8[:], mr[:, 0:NCT], mr[:, 0:NCT], mult)
            nc.vector.tensor_tensor(var8[:], tmp8[:], var8[:], sub)
            std8 = gn_sb.tile([G // 4, NCT], FP32, tag="std8")
            eps8 = gn_sb.tile([G // 4, 1], FP32, tag="eps8")
            nc.vector.memset(eps8[:], EPS)
            nc.scalar.activation(std8[:], var8[:], Sqrt, bias=eps8[:])
            nc.vector.reciprocal(mr[:, NCT:2 * NCT], std8[:])

            # broadcast mean/rstd to per-partition, apply GN affine on the
            # Pool engine (SBUF-only op: h = x * a + b via two scalars)
            mr_v = mr.rearrange("p (h f) -> p h f", h=2)
            for t in range(NCT):
                bc = ps_t.tile([128, 512], FP32, tag="tp", name=f"bc{t}")
                nc.tensor.matmul(bc[:, 0:2], jmat[:], mr_v[:, :, t])
                a_t = gn_sb.tile([128, 1], FP32, tag=f"a{t}", name=f"a{t}")
                b_t = gn_sb.tile([128, 1], FP32, tag=f"b{t}", name=f"b{t}")
                nc.vector.tensor_tensor(a_t[:], bc[:, 1:2], gns_t[:, t:t + 1],
                                        mult)
                # b = gn_bias - mean * a
                nc.vector.tensor_tensor(b_t[:], bc[:, 0:1], a_t[:], mult)
                nc.vector.tensor_tensor(b_t[:], gnb_t[:, t:t + 1], b_t[:], sub)
                # split the applies across DVE/ACT so h2 is ready in ~2
                # apply-times (Pool cannot run AP-scalar ops)
                h2_ap = h2[t // 2][:, (t % 2) * L:(t % 2 + 1) * L]
                if t % 2 == 0:
                    nc.vector.tensor_scalar(h2_ap, xtiles[t][:],
                                            a_t[:], b_t[:], mult, add)
                else:
                    nc.scalar.activation(h2_ap, xtiles[t][:], Id,
                                         bias=b_t[:], scale=a_t[:])

            transpose_weight(wq_d, "wq", wqT_pool, wq2T, nc.sync)
            transpose_weight(wk_d, "wk", wkT_pool, wk2T, nc.sync)
            transpose_weight(wv_d, "wv", wvT_pool, wv2T, nc.scalar)

            # ---- projections (all DoubleRow fp8: contract c-pairs).
            # Two adjacent 512-wide outputs accumulate into the two banks of
            # one [128,1024] PSUM tile; a single instruction evacuates both,
            # alternating DVE/ACT. ----
            for dst_list, w2T, bvec, pool, nmo in (
                    (qt2, wq2T, bq_t, qt_pool, "q"),
                    (kt2, wk2T, bk_t, kt_pool, "k")):
                for j in range(NPR):
                    dst_list.append(pool.tile([128, 2 * L], FP8, tag=pool.name,
                                              name=f"{nmo}T{j}"))
                for t in range(NCT):
                    half = (t % 2) * L
                    for lc2 in range(NQC // 2):
                        pp = ps_p.tile([128, 1024], FP32, tag="pp", name="pp")
                        for sub_i in range(2):
                            lc = 2 * lc2 + sub_i
                            for j in range(NPR):
                                nc.tensor.matmul(
                                    pp[:, 512 * sub_i:512 * (sub_i + 1)],
                                    _pair(w2T[j])[:, 0:2, 128 * t:128 * (t + 1)],
                                    _pair(h2[j])[:, 0:2,
                                                 512 * lc:512 * (lc + 1)],
                                    start=(j == 0), stop=(j == NPR - 1),
                                    perf_mode=DR, skip_group_check=True)
                        dst_ap = dst_list[t // 2][:, half + 1024 * lc2:
                                                  half + 1024 * (lc2 + 1)]
                        if (t * 2 + lc2) % 2 == 0:
                            nc.scalar.activation(dst_ap, pp[:], Id,
                                                 bias=bvec[:, t:t + 1])
                        else:
                            nc.vector.tensor_scalar(dst_ap, pp[:],
                                                    bvec[:, t:t + 1], None, add)

            # V: bv is NOT added here — a = (P(V + 1 bv^T))/d = PV/d + bv,
            # so bv folds into the out-projection bias: bo' = bo + wo @ bv
            # (computed below with two tiny DoubleRow matmuls per c-tile).
            for m in range(NLT // 2):
                vl2.append(vl_pool.tile([128, 2 * C], FP8, tag="vl",
                                        name=f"vl{m}"))
            for lt2 in range(NLT // 2):
                pp = ps_p.tile([128, 1024], FP32, tag="pp", name="pp")
                for sub_i in range(2):
                    lt = 2 * lt2 + sub_i
                    for j in range(NPR):
                        nc.tensor.matmul(
                            pp[:, 512 * sub_i:512 * (sub_i + 1)],
                            _pair(h2[j])[:, 0:2, 128 * lt:128 * (lt + 1)],
                            _pair(wv2T[j])[:, 0:2, :],
                            start=(j == 0), stop=(j == NPR - 1),
                            perf_mode=DR, skip_group_check=True)
                if lt2 % 2 == 0:
                    nc.scalar.copy(vl2[lt2][:], pp[:])
                else:
                    nc.vector.tensor_copy(vl2[lt2][:], pp[:])

            # wo^T is not needed until the first out-projection (~end of the
            # first attention q-chunk) — transpose it last
            transpose_weight(wo_d, "wo", wot_pool, wo2T, nc.scalar)

            # bo' = bo + wo @ bv  (8 tiny DoubleRow matmuls into one bank)
            bv8 = gn_sb.tile([128, NCT], FP8, tag="bv8")
            nc.vector.tensor_copy(bv8[:], bv_t[:])
            wob_ps = ps_gn.tile([128, NCT], FP32, tag="wob")
            for ot in range(NCT):
                for j in range(NPR):
                    bv_pair = bv8[:, 2 * j:2 * j + 2].rearrange(
                        "p (two f) -> p two f", two=2)
                    nc.tensor.matmul(
                        wob_ps[:, ot:ot + 1],
                        _pair(wo2T[j])[:, 0:2, 128 * ot:128 * (ot + 1)],
                        bv_pair[:, 0:2, :],
                        start=(ot == 0 and j == 0),
                        stop=(ot == NCT - 1 and j == NPR - 1),
                        perf_mode=DR, skip_group_check=True)
            bo2_t = vecs.tile([128, NCT], FP32, tag="bo2")
            nc.vector.tensor_tensor(bo2_t[:], wob_ps[:], bo_t[:], add)

        # ---- attention ----
        with ExitStack() as att_ctx:
            pt_pool = att_ctx.enter_context(tc.tile_pool(name="pt", bufs=6))
            dinv_pool = att_ctx.enter_context(tc.tile_pool(name="dinv", bufs=2))
            asb_pool = att_ctx.enter_context(tc.tile_pool(name="asb", bufs=4))
            osb_pool = att_ctx.enter_context(tc.tile_pool(name="osb", bufs=4))
            # 8 PSUM banks: 3 x S (two-deep prefetch) + 2x2 x A + 1 x d.
            # The out-projection o_ps tiles borrow freed A banks (same pool)
            # after the A evacuation each q-chunk.
            ps_s = att_ctx.enter_context(
                tc.tile_pool(name="ps_s", bufs=3, space="PSUM"))
            ps_a = att_ctx.enter_context(
                tc.tile_pool(name="ps_a", bufs=2, space="PSUM"))
            ps_d = att_ctx.enter_context(
                tc.tile_pool(name="ps_d", bufs=1, space="PSUM"))

            def s_block(qc_i, kt_i):
                s_tile = ps_s.tile([128, 512], FP32, tag="s",
                                   name=f"s{qc_i}_{kt_i}")
                for j in range(NPR):
                    nc.tensor.matmul(
                        s_tile[:],
                        _pair(kt2[j])[:, 0:2, 128 * kt_i:128 * (kt_i + 1)],
                        _pair(qt2[j])[:, 0:2, 512 * qc_i:512 * qc_i + 512],
                        start=(j == 0), stop=(j == NPR - 1), perf_mode=DR)
                return s_tile

            from collections import deque
            s_ahead = deque()
            s_ahead.append(s_block(0, 0))
            s_ahead.append(s_block(0, 1))
            for qc in range(NQC):
                q0 = 512 * qc
                # a_ps[j] holds c-tiles (2j, 2j+1) in its two banks
                a_ps = [ps_a.tile([128, 1024], FP32, tag="a", name=f"a_ps{j}")
                        for j in range(NPR)]
                d_ps = ps_d.tile([128, 512], FP32, tag="d", name="d_ps")
                p_cur = None
                for kt_i in range(NLT):
                    if kt_i % 2 == 0:
                        p_cur = pt_pool.tile([128, 1024], FP8, tag="pt",
                                             name="p_t")
                    s_cur = s_ahead.popleft()
                    nc.scalar.activation(
                        p_cur[:, (kt_i % 2) * 512:(kt_i % 2 + 1) * 512],
                        s_cur[:], Exp, scale=SCALE, bias=logp_t[:])
                    # keep two S blocks in flight ahead of the exp so ACT
                    # never waits on the PE
                    nxt = kt_i + 2
                    if nxt < NLT:
                        s_ahead.append(s_block(qc, nxt))
                    elif qc + 1 < NQC:
                        s_ahead.append(s_block(qc + 1, nxt - NLT))
                    if kt_i % 2 == 0:
                        continue
                    pair = kt_i // 2
                    first = pair == 0
                    last = pair == NLT // 2 - 1
                    pv = _pair(p_cur)
                    for cc in range(NCT):
                        nc.tensor.matmul(
                            a_ps[cc // 2][:, (cc % 2) * 512:(cc % 2 + 1) * 512],
                            _pair(vl2[pair])[:, 0:2, 128 * cc:128 * (cc + 1)],
                            pv[:, 0:2, :],
                            start=first, stop=last, skip_group_check=True,
                            perf_mode=DR)
                    # softmax denominator: ones-matmul accumulation (the
                    # result lands broadcast across all 128 partitions)
                    nc.tensor.matmul(d_ps[:], _pair(ones8)[:, 0:2, :],
                                     pv[:, 0:2, :],
                                     start=first, stop=last,
                                     skip_group_check=True, perf_mode=DR)

                # Evacuate UNNORMALIZED A (frees the accumulation banks
                # without waiting for the reciprocal); the softmax division
                # commutes with the out-projection (per-column scaling), so
                # it is applied at the final evacuation instead.  Both
                # merged copies on DVE — ACT stays dedicated to the exps.
                a2 = []
                for j in range(NPR):
                    t = asb_pool.tile([128, 1024], FP8, tag="asb",
                                      name=f"asb{j}")
                    # ACT takes one (it idles at the q-chunk boundary)
                    if j == 0:
                        nc.scalar.copy(t[:], a_ps[j][:])
                    else:
                        nc.vector.tensor_copy(t[:], a_ps[j][:])
                    a2.append(t)
                dinv = dinv_pool.tile([128, 512], FP32, tag="dinv", name="dinv")
                dscr = dinv_pool.tile([128, 512], FP32, tag="dscr", name="dscr")
                nc.vector.reciprocal_approx_accurate(out=dinv[:], in_=d_ps[:],
                                                     scratch=dscr[:])

                for op in range(NPR):  # two [128,1024] out tiles: ot pairs
                    o_ps = ps_a.tile([128, 1024], FP32, tag="a", name="o_ps")
                    for sub_i in range(2):
                        ot = 2 * op + sub_i
                        for j in range(NPR):
                            nc.tensor.matmul(
                                o_ps[:, 512 * sub_i:512 * (sub_i + 1)],
                                _pair(wo2T[j])[:, 0:2, 128 * ot:128 * (ot + 1)],
                                _pair(a2[j])[:, 0:2, :],
                                start=(j == 0), stop=(j == NPR - 1),
                                perf_mode=DR, skip_group_check=True)
                    osb = osb_pool.tile([128, 1024], FP32, tag="osb",
                                        name="osb")
                    for sub_i in range(2):
                        ot = 2 * op + sub_i
                        sl = slice(512 * sub_i, 512 * (sub_i + 1))
                        tmp = osb_pool.tile([128, 512], FP32, tag="otmp",
                                            name="otmp")
                        nc.vector.tensor_tensor(tmp[:], o_ps[:, sl],
                                                dinv[:], mult)
                        nc.vector.scalar_tensor_tensor(
                            osb[:, sl], tmp[:], bo2_t[:, ot:ot + 1],
                            xtiles[ot][:, q0:q0 + 512],
                            op0=add, op1=add)
                    # one DMA for both ot row-blocks (256 DRAM rows);
                    # alternate the two queues
                    dma_eng = nc.scalar if op % 2 == 0 else nc.sync
                    dma_eng.dma_start(
                        out=out_d[256 * op:256 * (op + 1), q0:q0 + 512]
                        .rearrange("(two p) c -> p two c", p=128),
                        in_=osb[:].rearrange("p (two c) -> p two c", two=2))


def make_nc():
    return bacc.Bacc("TRN2", target_bir_lowering=False, debug=False)


_NC_CACHE = []


def kernel(**inputs):
    from concourse.bass_utils import run_bass_kernel_spmd

    x = np.ascontiguousarray(inputs["x"], dtype=np.float32)
    B = x.shape[0]
    assert B == 8, f"kernel is built for B=8 (one batch element per core), got {B}"
    shared = {}
    for name in ("gn_scale", "gn_bias", "wq", "bq", "wk", "bk",
                 "wv", "bv", "wo", "bo"):
        shared[name] = np.ascontiguousarray(inputs[name], dtype=np.float32)

    if not _NC_CACHE:
        nc = make_nc()
        build_kernel(nc)
        nc.compile()
        _NC_CACHE.append(nc)
    nc = _NC_CACHE[0]

    core_ids = list(range(B))
    in_maps = [dict(shared, x=x[i]) for i in range(B)]
    res = run_bass_kernel_spmd(nc, in_maps, core_ids)
    out = np.stack([res.results[i]["out"] for i in range(B)], axis=0)
    return out.astype(np.float32)


if __name__ == "__main__":
    rng = np.random.default_rng(0)
    demo = {
        "x": rng.standard_normal((8, C, L), dtype=np.float32),
        "gn_scale": np.ones(C, np.float32),
        "gn_bias": np.zeros(C, np.float32),
    }
    for w, b in (("wq", "bq"), ("wk", "bk"), ("wv", "bv"), ("wo", "bo")):
        demo[w] = rng.standard_normal((C, C), dtype=np.float32) / np.sqrt(C)
        demo[b] = np.zeros(C, np.float32)
    out = kernel(**demo)
    print(out.shape, out.dtype)
